# revision 19
# baseline (speedup 1.0000x reference)
"""Trainium2 Bass kernel for nn_NoiseLayer (segment_reduce).

Computes, for x:[16384,2048] f32 and y:[16384] int class labels (C=1000):
    per-class mean/std over rows of x  (segment reduction),
    class_noise = mean + std * N(0,1)  (fixed jax PRNG, key 42),
    newY = y[perm]                      (fixed jax PRNG permutation),
    out  = 0.9*x + 0.1*class_noise[newY]

Distribution strategy: columns of x are sharded 8 ways (256 cols/core), so
each core sees ALL rows of its column shard and the segment reduction is
fully local -- no collectives.  Rows are presorted on the host:
  phase A consumes rows sorted by y     -> banded one-hot matmuls give
                                           per-class sum / sum-of-squares,
  phase B consumes rows sorted by newY  -> banded one-hot matmuls broadcast
                                           noise rows back to row space.
All index/permutation metadata is tiny host-side work; all heavy data
movement and math runs on the NeuronCores.
"""

import os
import subprocess
import sys
import tempfile
import time

import numpy as np


def _log(msg):
    print(f"[kernel +{time.time() - _T0:.1f}s] {msg}", file=sys.stderr, flush=True)


_T0 = time.time()

B = 16384
D = 2048
C = 1000
CP = 1024  # C padded to a multiple of 128
NCORES = 8
DS = D // NCORES  # 256 columns per core
NT = B // 128  # 128 row-tiles
NCH = NT // 8  # 16 chunks of 8 row-tiles
ALPHA = np.float32(0.1)
ONE_MINUS_ALPHA = np.float32(1.0) - ALPHA

# "bf16": stats matmuls in bf16 (full PE speed; x is rounded to bf16 for the
#         sum/sum-of-squares only -- the dominant 0.9*x output term stays f32).
# "f32":  stats matmuls in fp32 (bit-accurate sums, 4x slower on PE).
STATS_MODE = "bf16"

PROFILE = False  # test.py sets this to capture an NTFF profile
LAST_RESULTS = None  # BassKernelResults of the most recent run (for test.py)

_CONSTS_SCRIPT = r"""
import numpy as np, jax
nk, pk = jax.random.split(jax.random.key(42))
noise = jax.random.normal(nk, (%d, %d), jax.numpy.float32)
index = jax.random.permutation(pk, %d)
np.savez(%r, noise=np.asarray(noise), index=np.asarray(index))
""".strip()

_consts_cache = None


def _host_constants():
    """noise:[C,D] f32 and index:[B] from jax PRNG, bit-exact vs reference.

    Runs in a scrubbed-env subprocess on the jax CPU backend: the parent
    process environment forces the axon/neuron PJRT platform, where each jax
    op is compiled by neuronx-cc (minutes).  Threefry PRNG results are
    backend-independent, so CPU output matches the reference exactly.
    """
    global _consts_cache
    if _consts_cache is not None:
        return _consts_cache
    with tempfile.TemporaryDirectory() as td:
        path = os.path.join(td, "consts.npz")
        env = dict(os.environ)
        env.pop("TRN_TERMINAL_POOL_IPS", None)  # skip the axon boot hook
        env["JAX_PLATFORMS"] = "cpu"
        env["PYTHONPATH"] = ""
        script = _CONSTS_SCRIPT % (C, D, B, path)
        subprocess.run(
            [sys.executable, "-c", script], env=env, check=True, capture_output=True
        )
        data = np.load(path)
        _consts_cache = (data["noise"].copy(), data["index"].copy())
    return _consts_cache


def _build_program(pairs_a, pairs_b, n_a, n_b):
    """Build the SPMD Bass/Tile program (identical on all 8 cores).

    pairs_a: per M-tile (8 class-blocks of 128) the list of 128-row K-tiles
             overlapping that class range in y-sorted order.
    pairs_b: per row-tile (newY-sorted) the list of 128-class blocks its
             newY values fall into.
    """
    import concourse.bacc as bacc
    import concourse.bass as bass
    import concourse.mybir as mybir
    import concourse.tile as tile
    from contextlib import ExitStack

    f32 = mybir.dt.float32
    f32r = mybir.dt.float32r
    bf16 = mybir.dt.bfloat16
    op = mybir.AluOpType

    nc = bacc.Bacc("TRN2", target_bir_lowering=False, debug=False, num_devices=NCORES)

    # all f32 constants packed in one tensor (one DMA -> one wait semaphore):
    # [iotaf:128 | ysm:n_a | scal_a:8 | scal_e:8 | nconst:8*DS]
    W = 128 + n_a + 8 + 8 + 8 * DS
    xs1dt = bf16 if STATS_MODE == "bf16" else f32
    xs1 = nc.dram_tensor("xs1", [B, DS], xs1dt, kind="ExternalInput").ap()
    xs2 = nc.dram_tensor("xs2", [B, DS], f32, kind="ExternalInput").ap()
    cst = nc.dram_tensor("cst", [128, W], f32, kind="ExternalInput").ap()
    oh2 = nc.dram_tensor("oh2", [128, n_b * 128], bf16, kind="ExternalInput").ap()
    outp = nc.dram_tensor("outp", [B, DS], f32, kind="ExternalOutput").ap()

    # row r = c*1024 + j*128 + p  ->  chunk c, SBUF partition p, slot j
    xs1_r = xs1.rearrange("(c j p) d -> c p j d", c=NCH, j=8, p=128)
    xs2_r = xs2.rearrange("(c j p) d -> c p j d", c=NCH, j=8, p=128)
    out_r = outp.rearrange("(c j p) d -> c p j d", c=NCH, j=8, p=128)

    mmdt = bf16 if STATS_MODE == "bf16" else f32

    with tile.TileContext(nc) as tc:
        with ExitStack() as ctx:
            consts = ctx.enter_context(tc.tile_pool(name="consts", bufs=1))
            xs1p = ctx.enter_context(tc.tile_pool(name="xs1p", bufs=3))
            sqp = ctx.enter_context(tc.tile_pool(name="sqp", bufs=3))
            ohp = ctx.enter_context(tc.tile_pool(name="ohp", bufs=4))
            psa = ctx.enter_context(
                tc.tile_pool(name="psa", bufs=2, space=bass.MemorySpace.PSUM)
            )
            stat = ctx.enter_context(tc.tile_pool(name="stat", bufs=2))
            xs2p = ctx.enter_context(tc.tile_pool(name="xs2p", bufs=6))
            psb = ctx.enter_context(
                tc.tile_pool(name="psb", bufs=4, space=bass.MemorySpace.PSUM)
            )
            outsp = ctx.enter_context(tc.tile_pool(name="outsp", bufs=3))

            # ---- constants (single DMA for all f32 constants) ----
            cst_t = consts.tile([128, W], f32)
            nc.sync.dma_start(cst_t[:], cst[:])
            iotaf_t = cst_t[:, 0:128]
            ysm_t = cst_t[:, 128 : 128 + n_a]
            sa_t = cst_t[:, 128 + n_a : 128 + n_a + 8]
            se_t = cst_t[:, 128 + n_a + 8 : 128 + n_a + 16]
            ncst_t = cst_t[:, 128 + n_a + 16 :].rearrange("p (m d) -> p m d", m=8)
            oh2_t = consts.tile([128, n_b * 128], bf16)
            nc.sync.dma_start(oh2_t[:], oh2[:])
            # noise table, class c -> partition c%128, block c//128 (bf16)
            noiseb = consts.tile([128, 8, DS], bf16)

            # ---- phase A: per-class sum & sum-of-squares via banded matmul ----
            chunk_tiles = {}

            def get_chunk(c):
                if c not in chunk_tiles:
                    xt = xs1p.tile([128, 8, DS], xs1dt, tag="xc", name=f"xc{c}")
                    nc.sync.dma_start(xt[:], xs1_r[c])
                    st = sqp.tile([128, 8, DS], mmdt, tag="sc", name=f"sc{c}")
                    nc.scalar.square(st[:], xt[:])
                    chunk_tiles[c] = (xt, st)
                return chunk_tiles[c]

            ja = 0
            for m in range(8):
                trange = pairs_a[m]
                if not trange:
                    nc.vector.memset(noiseb[:, m, :], 0.0)
                    continue
                ps_s = psa.tile([128, DS], f32, tag="ps_s", name=f"ps_s{m}")
                ps_q = psa.tile([128, DS], f32, tag="ps_q", name=f"ps_q{m}")
                for i, t in enumerate(trange):
                    xm, st = get_chunk(t // 8)
                    oh = ohp.tile([128, 128], mmdt, tag="oh", name=f"oh{m}_{t}")
                    # tensor_tensor with a free-dim-broadcast AP instead of
                    # tensor_scalar: the TensorScalarPtr encoding has too few
                    # sync-wait slots for Tile's semaphores (walrus rejects).
                    nc.vector.tensor_tensor(
                        oh[:],
                        iotaf_t[:],
                        ysm_t[:, ja : ja + 1].broadcast_to((128, 128)),
                        op.is_equal,
                    )
                    first, last = i == 0, i == len(trange) - 1
                    nc.tensor.matmul(
                        ps_s[:], oh[:], xm[:, t % 8, :], start=first, stop=last
                    )
                    nc.tensor.matmul(
                        ps_q[:], oh[:], st[:, t % 8, :], start=first, stop=last
                    )
                    ja += 1
                # stats: u=mean, var=(sq - u*s)/(cnt-1) clamped, noise=u+std*n
                u = stat.tile([128, DS], f32, tag="u", name=f"u{m}")
                nc.vector.tensor_tensor(
                    u[:], ps_s[:], sa_t[:, m : m + 1].broadcast_to((128, DS)), op.mult
                )
                w2 = stat.tile([128, DS], f32, tag="w2", name=f"w2{m}")
                nc.vector.tensor_tensor(w2[:], u[:], ps_s[:], op.mult)
                dd = stat.tile([128, DS], f32, tag="dd", name=f"dd{m}")
                nc.vector.tensor_tensor(dd[:], ps_q[:], w2[:], op.subtract)
                var = stat.tile([128, DS], f32, tag="var", name=f"var{m}")
                nc.vector.tensor_tensor(
                    var[:], dd[:], se_t[:, m : m + 1].broadcast_to((128, DS)), op.mult
                )
                varc = stat.tile([128, DS], f32, tag="varc", name=f"varc{m}")
                nc.vector.tensor_relu(varc[:], var[:])
                std = stat.tile([128, DS], f32, tag="std", name=f"std{m}")
                nc.scalar.sqrt(std[:], varc[:])
                sn = stat.tile([128, DS], f32, tag="sn", name=f"sn{m}")
                nc.vector.tensor_tensor(sn[:], std[:], ncst_t[:, m, :], op.mult)
                nc.vector.tensor_tensor(noiseb[:, m, :], sn[:], u[:], op.add)

            # ---- phase B: out = 0.9*x + one_hot(newY) @ (0.1*noise) ----
            # (the 0.1 scale is folded into the host-built one-hot tiles,
            #  the 0.9 scale is folded into the host-prepared xs2)
            jb = 0
            for c in range(NCH):
                x2 = xs2p.tile([128, 8, DS], f32, tag="x2", name=f"x2_{c}")
                nc.sync.dma_start(x2[:], xs2_r[c])
                ot = outsp.tile([128, 8, DS], f32, tag="ot", name=f"ot{c}")
                for j in range(8):
                    rt = c * 8 + j
                    blocks = pairs_b[rt]
                    pb = psb.tile([128, DS], f32, tag="pb", name=f"pb{rt}")
                    for i, bb in enumerate(blocks):
                        first, last = i == 0, i == len(blocks) - 1
                        nc.tensor.matmul(
                            pb[:],
                            oh2_t[:, jb * 128 : (jb + 1) * 128],
                            noiseb[:, bb, :],
                            start=first,
                            stop=last,
                        )
                        jb += 1
                    nc.vector.tensor_tensor(ot[:, j, :], pb[:], x2[:, j, :], op.add)
                nc.sync.dma_start(out_r[c], ot[:])

    nc.compile()
    return nc


def kernel(x, y):
    global LAST_RESULTS, _T0
    _T0 = time.time()
    x = np.ascontiguousarray(np.asarray(x), dtype=np.float32)
    y = np.asarray(y)
    assert x.shape == (B, D) and y.shape == (B,)
    _log("host prep start")

    import ml_dtypes
    from concourse.bass_utils import run_bass_kernel_spmd

    noise, index = _host_constants()
    newY = y[index]

    yi = y.astype(np.int64)
    cnt = np.bincount(yi, minlength=CP).astype(np.float64)

    perm1 = np.argsort(yi, kind="stable")
    y_s = yi[perm1]
    perm2 = np.argsort(newY.astype(np.int64), kind="stable")
    newY2 = newY.astype(np.int64)[perm2]

    # phase A band structure (which 128-row K-tiles feed each class block)
    bounds = np.searchsorted(y_s, np.arange(0, CP + 1, 128))
    pairs_a = []
    ysm_cols = []
    for m in range(8):
        r0, r1 = int(bounds[m]), int(bounds[m + 1])
        if r0 == r1:
            pairs_a.append([])
            continue
        trange = list(range(r0 // 128, (r1 + 127) // 128))
        pairs_a.append(trange)
        for t in trange:
            ysm_cols.append(y_s[128 * t : 128 * t + 128].astype(np.float32) - 128.0 * m)
    n_a = len(ysm_cols)
    ysm_np = np.stack(ysm_cols, axis=1) if n_a else np.zeros((128, 1), np.float32)
    n_a = max(n_a, 1)

    # phase B band structure (which class blocks each newY-sorted row-tile hits)
    pairs_b = []
    oh2_blocks = []
    karange = np.arange(128)
    for rt in range(NT):
        seg = newY2[128 * rt : 128 * rt + 128]
        lo, hi = int(seg[0]) // 128, int(seg[-1]) // 128
        blocks = list(range(lo, hi + 1))
        pairs_b.append(blocks)
        for bb in blocks:
            ohm = (seg[None, :] == (128 * bb + karange)[:, None]).astype(np.float32)
            oh2_blocks.append(ohm * ALPHA)
    n_b = len(oh2_blocks)
    oh2_np = (
        np.concatenate(oh2_blocks, axis=1).astype(ml_dtypes.bfloat16)
        if n_b
        else np.zeros((128, 128), ml_dtypes.bfloat16)
    )
    n_b = max(n_b, 1)

    # per-class scalars, padded classes get cnt=1 so everything stays finite
    cnt_safe = np.where(cnt > 0, cnt, 1.0)
    a_np = (1.0 / cnt_safe).astype(np.float32)
    e_np = (1.0 / np.maximum(cnt_safe - 1.0, 1.0)).astype(np.float32)
    # [p, m] = value for class 128*m + p
    a_np = a_np.reshape(8, 128).T.copy()
    e_np = e_np.reshape(8, 128).T.copy()

    iotaf_np = np.broadcast_to(np.arange(128, dtype=np.float32), (128, 128)).copy()

    noise_pad = np.zeros((CP, D), np.float32)
    noise_pad[:C] = noise

    # heavy host prep: the two row orderings (0.9 folded into the phase-B copy)
    x_s1 = x[perm1]
    x_s2 = x[perm2] * ONE_MINUS_ALPHA

    _log("building bass program")
    nc = _build_program(pairs_a, pairs_b, n_a, n_b)
    _log("program built")

    in_maps = []
    for core in range(NCORES):
        cs = slice(core * DS, (core + 1) * DS)
        nconst_np = (
            noise_pad[:, cs].reshape(8, 128, DS).transpose(1, 0, 2).reshape(128, 8 * DS)
        )
        cst_np = np.ascontiguousarray(
            np.concatenate([iotaf_np, ysm_np, a_np, e_np, nconst_np], axis=1),
            dtype=np.float32,
        )
        xs1_core = np.ascontiguousarray(x_s1[:, cs])
        if STATS_MODE == "bf16":
            xs1_core = xs1_core.astype(ml_dtypes.bfloat16)
        in_maps.append(
            {
                "xs1": xs1_core,
                "xs2": np.ascontiguousarray(x_s2[:, cs]),
                "cst": cst_np,
                "oh2": oh2_np,
            }
        )

    _log("launching spmd run (compile + transfer + execute)")
    res = run_bass_kernel_spmd(
        nc, in_maps, list(range(NCORES)), trace=PROFILE
    )
    LAST_RESULTS = res
    _log("spmd run done")

    out_s2 = np.concatenate([res.results[i]["outp"] for i in range(NCORES)], axis=1)
    out = np.empty((B, D), np.float32)
    out[perm2] = out_s2
    return out, newY


# revision 24
# speedup vs baseline: 1.1190x; 1.1190x over previous
"""Trainium2 Bass kernel for nn_NoiseLayer (segment_reduce).

Computes, for x:[16384,2048] f32 and y:[16384] int class labels (C=1000):
    per-class mean/std over rows of x  (segment reduction),
    class_noise = mean + std * N(0,1)  (fixed jax PRNG, key 42),
    newY = y[perm]                      (fixed jax PRNG permutation),
    out  = 0.9*x + 0.1*class_noise[newY]

Distribution strategy: columns of x are sharded 8 ways (256 cols/core), so
each core sees ALL rows of its column shard and the segment reduction is
fully local -- no collectives.  Rows are presorted on the host:
  phase A consumes rows sorted by y     -> banded one-hot matmuls give
                                           per-class sum / sum-of-squares,
  phase B consumes rows sorted by newY  -> banded one-hot matmuls broadcast
                                           noise rows back to row space.
All index/permutation metadata is tiny host-side work; all heavy data
movement and math runs on the NeuronCores.
"""

import os
import subprocess
import sys
import tempfile
import time

import numpy as np


def _log(msg):
    print(f"[kernel +{time.time() - _T0:.1f}s] {msg}", file=sys.stderr, flush=True)


_T0 = time.time()

B = 16384
D = 2048
C = 1000
CP = 1024  # C padded to a multiple of 128
NCORES = 8
DS = D // NCORES  # 256 columns per core
NT = B // 128  # 128 row-tiles
NCH = NT // 8  # 16 chunks of 8 row-tiles
ALPHA = np.float32(0.1)
ONE_MINUS_ALPHA = np.float32(1.0) - ALPHA

# "bf16": stats matmuls in bf16 (full PE speed; x is rounded to bf16 for the
#         sum/sum-of-squares only -- the dominant 0.9*x output term stays f32).
# "f32":  stats matmuls in fp32 (bit-accurate sums, 4x slower on PE).
STATS_MODE = "bf16"

PROFILE = False  # test.py sets this to capture an NTFF profile
LAST_RESULTS = None  # BassKernelResults of the most recent run (for test.py)

_CONSTS_SCRIPT = r"""
import numpy as np, jax
nk, pk = jax.random.split(jax.random.key(42))
noise = jax.random.normal(nk, (%d, %d), jax.numpy.float32)
index = jax.random.permutation(pk, %d)
np.savez(%r, noise=np.asarray(noise), index=np.asarray(index))
""".strip()

_consts_cache = None


def _host_constants():
    """noise:[C,D] f32 and index:[B] from jax PRNG, bit-exact vs reference.

    Runs in a scrubbed-env subprocess on the jax CPU backend: the parent
    process environment forces the axon/neuron PJRT platform, where each jax
    op is compiled by neuronx-cc (minutes).  Threefry PRNG results are
    backend-independent, so CPU output matches the reference exactly.
    """
    global _consts_cache
    if _consts_cache is not None:
        return _consts_cache
    with tempfile.TemporaryDirectory() as td:
        path = os.path.join(td, "consts.npz")
        env = dict(os.environ)
        env.pop("TRN_TERMINAL_POOL_IPS", None)  # skip the axon boot hook
        env["JAX_PLATFORMS"] = "cpu"
        env["PYTHONPATH"] = ""
        script = _CONSTS_SCRIPT % (C, D, B, path)
        subprocess.run(
            [sys.executable, "-c", script], env=env, check=True, capture_output=True
        )
        data = np.load(path)
        _consts_cache = (data["noise"].copy(), data["index"].copy())
    return _consts_cache


def _build_program(pairs_a, pairs_b, n_a, n_b):
    """Build the SPMD Bass/Tile program (identical on all 8 cores).

    pairs_a: per M-tile (8 class-blocks of 128) the list of 128-row K-tiles
             overlapping that class range in y-sorted order.
    pairs_b: per row-tile (newY-sorted) the list of 128-class blocks its
             newY values fall into.
    """
    import concourse.bacc as bacc
    import concourse.bass as bass
    import concourse.mybir as mybir
    import concourse.tile as tile
    from contextlib import ExitStack

    f32 = mybir.dt.float32
    f32r = mybir.dt.float32r
    bf16 = mybir.dt.bfloat16
    op = mybir.AluOpType

    nc = bacc.Bacc("TRN2", target_bir_lowering=False, debug=False, num_devices=NCORES)

    # all f32 constants packed in one tensor (one DMA -> one wait semaphore):
    # [iotaf:128 | ysm:n_a | scal_a:8 | scal_e:8 | nconst:8*DS]
    W = 128 + n_a + 8 + 8 + 8 * DS
    xs1dt = bf16 if STATS_MODE == "bf16" else f32
    # x copies are host-packed to the on-chip layout [chunk, partition, j, d]
    # (row r = c*1024 + j*128 + p) so every DMA descriptor moves a contiguous
    # 4-8KB per partition instead of 1KB rows.
    xs1 = nc.dram_tensor("xs1", [NCH, 128, 8, DS], xs1dt, kind="ExternalInput").ap()
    xs2 = nc.dram_tensor("xs2", [NCH, 128, 8, DS], f32, kind="ExternalInput").ap()
    cst = nc.dram_tensor("cst", [128, W], f32, kind="ExternalInput").ap()
    oh2 = nc.dram_tensor("oh2", [128, n_b * 128], bf16, kind="ExternalInput").ap()
    outp = nc.dram_tensor("outp", [NCH, 128, 8, DS], f32, kind="ExternalOutput").ap()

    xs1_r = xs1
    xs2_r = xs2
    out_r = outp

    mmdt = bf16 if STATS_MODE == "bf16" else f32

    with tile.TileContext(nc) as tc:
        with ExitStack() as ctx:
            consts = ctx.enter_context(tc.tile_pool(name="consts", bufs=1))
            xs1p = ctx.enter_context(tc.tile_pool(name="xs1p", bufs=3))
            sqp = ctx.enter_context(tc.tile_pool(name="sqp", bufs=3))
            ohp = ctx.enter_context(tc.tile_pool(name="ohp", bufs=4))
            psa = ctx.enter_context(
                tc.tile_pool(name="psa", bufs=2, space=bass.MemorySpace.PSUM)
            )
            stat = ctx.enter_context(tc.tile_pool(name="stat", bufs=2))
            xs2p = ctx.enter_context(tc.tile_pool(name="xs2p", bufs=6))
            psb = ctx.enter_context(
                tc.tile_pool(name="psb", bufs=2, space=bass.MemorySpace.PSUM)
            )
            outsp = ctx.enter_context(tc.tile_pool(name="outsp", bufs=3))

            # ---- constants (single DMA for all f32 constants) ----
            cst_t = consts.tile([128, W], f32)
            nc.sync.dma_start(cst_t[:], cst[:])
            iotaf_t = cst_t[:, 0:128]
            ysm_t = cst_t[:, 128 : 128 + n_a]
            sa_t = cst_t[:, 128 + n_a : 128 + n_a + 8]
            se_t = cst_t[:, 128 + n_a + 8 : 128 + n_a + 16]
            ncst_t = cst_t[:, 128 + n_a + 16 :].rearrange("p (m d) -> p m d", m=8)
            oh2_t = consts.tile([128, n_b * 128], bf16)
            nc.sync.dma_start(oh2_t[:], oh2[:])
            # noise table, class c -> partition c%128, block c//128 (bf16)
            noiseb = consts.tile([128, 8, DS], bf16)

            # ---- phase A: per-class sum & sum-of-squares via banded matmul ----
            chunk_tiles = {}

            def get_chunk(c):
                if c not in chunk_tiles:
                    xt = xs1p.tile([128, 8, DS], xs1dt, tag="xc", name=f"xc{c}")
                    nc.sync.dma_start(xt[:], xs1_r[c])
                    st = sqp.tile([128, 8, DS], mmdt, tag="sc", name=f"sc{c}")
                    nc.scalar.square(st[:], xt[:])
                    chunk_tiles[c] = (xt, st)
                return chunk_tiles[c]

            ja = 0
            for m in range(8):
                trange = pairs_a[m]
                if not trange:
                    nc.vector.memset(noiseb[:, m, :], 0.0)
                    continue
                ps_s = psa.tile([128, DS], f32, tag="ps_s", name=f"ps_s{m}")
                ps_q = psa.tile([128, DS], f32, tag="ps_q", name=f"ps_q{m}")
                for i, t in enumerate(trange):
                    xm, st = get_chunk(t // 8)
                    oh = ohp.tile([128, 128], mmdt, tag="oh", name=f"oh{m}_{t}")
                    # tensor_tensor with a free-dim-broadcast AP instead of
                    # tensor_scalar: the TensorScalarPtr encoding has too few
                    # sync-wait slots for Tile's semaphores (walrus rejects).
                    nc.vector.tensor_tensor(
                        oh[:],
                        iotaf_t[:],
                        ysm_t[:, ja : ja + 1].broadcast_to((128, 128)),
                        op.is_equal,
                    )
                    first, last = i == 0, i == len(trange) - 1
                    nc.tensor.matmul(
                        ps_s[:], oh[:], xm[:, t % 8, :], start=first, stop=last
                    )
                    nc.tensor.matmul(
                        ps_q[:], oh[:], st[:, t % 8, :], start=first, stop=last
                    )
                    ja += 1
                # stats: u=mean, var=(sq - u*s)/(cnt-1) clamped, noise=u+std*n
                u = stat.tile([128, DS], f32, tag="u", name=f"u{m}")
                nc.vector.tensor_tensor(
                    u[:], ps_s[:], sa_t[:, m : m + 1].broadcast_to((128, DS)), op.mult
                )
                w2 = stat.tile([128, DS], f32, tag="w2", name=f"w2{m}")
                nc.vector.tensor_tensor(w2[:], u[:], ps_s[:], op.mult)
                dd = stat.tile([128, DS], f32, tag="dd", name=f"dd{m}")
                nc.vector.tensor_tensor(dd[:], ps_q[:], w2[:], op.subtract)
                var = stat.tile([128, DS], f32, tag="var", name=f"var{m}")
                nc.vector.tensor_tensor(
                    var[:], dd[:], se_t[:, m : m + 1].broadcast_to((128, DS)), op.mult
                )
                varc = stat.tile([128, DS], f32, tag="varc", name=f"varc{m}")
                nc.vector.tensor_relu(varc[:], var[:])
                std = stat.tile([128, DS], f32, tag="std", name=f"std{m}")
                nc.scalar.sqrt(std[:], varc[:])
                sn = stat.tile([128, DS], f32, tag="sn", name=f"sn{m}")
                nc.vector.tensor_tensor(sn[:], std[:], ncst_t[:, m, :], op.mult)
                nc.vector.tensor_tensor(noiseb[:, m, :], sn[:], u[:], op.add)

            # ---- phase B: out = 0.9*x + one_hot(newY) @ (0.1*noise) ----
            # (the 0.1 scale is folded into the host-built one-hot tiles,
            #  the 0.9 scale is folded into the host-prepared xs2)
            jb = 0
            for c in range(NCH):
                x2 = xs2p.tile([128, 8, DS], f32, tag="x2", name=f"x2_{c}")
                nc.sync.dma_start(x2[:], xs2_r[c])
                ot = outsp.tile([128, 8, DS], f32, tag="ot", name=f"ot{c}")
                for half in range(2):
                    pb = psb.tile([128, 4, DS], f32, tag="pb", name=f"pb{c}_{half}")
                    for j4 in range(4):
                        rt = c * 8 + half * 4 + j4
                        blocks = pairs_b[rt]
                        for i, bb in enumerate(blocks):
                            first, last = i == 0, i == len(blocks) - 1
                            nc.tensor.matmul(
                                pb[:, j4, :],
                                oh2_t[:, jb * 128 : (jb + 1) * 128],
                                noiseb[:, bb, :],
                                start=first,
                                stop=last,
                            )
                            jb += 1
                    js = slice(half * 4, half * 4 + 4)
                    nc.vector.tensor_tensor(ot[:, js, :], pb[:], x2[:, js, :], op.add)
                nc.sync.dma_start(out_r[c], ot[:])

    nc.compile()
    return nc


def kernel(x, y):
    global LAST_RESULTS, _T0
    _T0 = time.time()
    x = np.ascontiguousarray(np.asarray(x), dtype=np.float32)
    y = np.asarray(y)
    assert x.shape == (B, D) and y.shape == (B,)
    _log("host prep start")

    import ml_dtypes
    from concourse.bass_utils import run_bass_kernel_spmd

    noise, index = _host_constants()
    newY = y[index]

    yi = y.astype(np.int64)
    cnt = np.bincount(yi, minlength=CP).astype(np.float64)

    perm1 = np.argsort(yi, kind="stable")
    y_s = yi[perm1]
    perm2 = np.argsort(newY.astype(np.int64), kind="stable")
    newY2 = newY.astype(np.int64)[perm2]

    # phase A band structure (which 128-row K-tiles feed each class block)
    bounds = np.searchsorted(y_s, np.arange(0, CP + 1, 128))
    pairs_a = []
    ysm_cols = []
    for m in range(8):
        r0, r1 = int(bounds[m]), int(bounds[m + 1])
        if r0 == r1:
            pairs_a.append([])
            continue
        trange = list(range(r0 // 128, (r1 + 127) // 128))
        pairs_a.append(trange)
        for t in trange:
            ysm_cols.append(y_s[128 * t : 128 * t + 128].astype(np.float32) - 128.0 * m)
    n_a = len(ysm_cols)
    ysm_np = np.stack(ysm_cols, axis=1) if n_a else np.zeros((128, 1), np.float32)
    n_a = max(n_a, 1)

    # phase B band structure (which class blocks each newY-sorted row-tile hits)
    pairs_b = []
    oh2_blocks = []
    karange = np.arange(128)
    for rt in range(NT):
        seg = newY2[128 * rt : 128 * rt + 128]
        lo, hi = int(seg[0]) // 128, int(seg[-1]) // 128
        blocks = list(range(lo, hi + 1))
        pairs_b.append(blocks)
        for bb in blocks:
            ohm = (seg[None, :] == (128 * bb + karange)[:, None]).astype(np.float32)
            oh2_blocks.append(ohm * ALPHA)
    n_b = len(oh2_blocks)
    oh2_np = (
        np.concatenate(oh2_blocks, axis=1).astype(ml_dtypes.bfloat16)
        if n_b
        else np.zeros((128, 128), ml_dtypes.bfloat16)
    )
    n_b = max(n_b, 1)

    # per-class scalars, padded classes get cnt=1 so everything stays finite
    cnt_safe = np.where(cnt > 0, cnt, 1.0)
    a_np = (1.0 / cnt_safe).astype(np.float32)
    e_np = (1.0 / np.maximum(cnt_safe - 1.0, 1.0)).astype(np.float32)
    # [p, m] = value for class 128*m + p
    a_np = a_np.reshape(8, 128).T.copy()
    e_np = e_np.reshape(8, 128).T.copy()

    iotaf_np = np.broadcast_to(np.arange(128, dtype=np.float32), (128, 128)).copy()

    noise_pad = np.zeros((CP, D), np.float32)
    noise_pad[:C] = noise

    # heavy host prep: the two row orderings (0.9 folded into the phase-B copy)
    x_s1 = x[perm1]
    x_s2 = x[perm2] * ONE_MINUS_ALPHA

    _log("building bass program")
    nc = _build_program(pairs_a, pairs_b, n_a, n_b)
    _log("program built")

    in_maps = []
    for core in range(NCORES):
        cs = slice(core * DS, (core + 1) * DS)
        nconst_np = (
            noise_pad[:, cs].reshape(8, 128, DS).transpose(1, 0, 2).reshape(128, 8 * DS)
        )
        cst_np = np.ascontiguousarray(
            np.concatenate([iotaf_np, ysm_np, a_np, e_np, nconst_np], axis=1),
            dtype=np.float32,
        )
        # pack to [chunk, partition, j, d] (row r = c*1024 + j*128 + p)
        xs1_core = (
            x_s1[:, cs].reshape(NCH, 8, 128, DS).transpose(0, 2, 1, 3)
        )
        if STATS_MODE == "bf16":
            xs1_core = xs1_core.astype(ml_dtypes.bfloat16)
        xs2_core = x_s2[:, cs].reshape(NCH, 8, 128, DS).transpose(0, 2, 1, 3)
        in_maps.append(
            {
                "xs1": np.ascontiguousarray(xs1_core),
                "xs2": np.ascontiguousarray(xs2_core),
                "cst": cst_np,
                "oh2": oh2_np,
            }
        )

    _log("launching spmd run (compile + transfer + execute)")
    res = run_bass_kernel_spmd(
        nc, in_maps, list(range(NCORES)), trace=PROFILE
    )
    LAST_RESULTS = res
    _log("spmd run done")

    out_s2 = np.concatenate(
        [
            res.results[i]["outp"].transpose(0, 2, 1, 3).reshape(B, DS)
            for i in range(NCORES)
        ],
        axis=1,
    )
    out = np.empty((B, D), np.float32)
    out[perm2] = out_s2
    _log("output assembled")
    return out, newY
